# revision 5
# baseline (speedup 1.0000x reference)
"""AttentionBlock (GroupNorm + 4-head self-attention + proj + residual) on 8
Trainium2 NeuronCores.

Sharding: core c handles image b = c//2 and head-pair hp = c%2 (heads
2*hp, 2*hp+1, i.e. a contiguous 128-channel block of each of q/k/v).
Each core computes GroupNorm stats for its image, folds them into the qkv
weights, runs flash-style attention for its two heads (no max subtraction;
scores are ~N(0,1) so exp never overflows), projects through its 128-column
block of proj_w, and returns a partial (256, 4096) output.  Host sums the
two partials per image and adds the residual x and proj bias.

All matmuls run in float32r (full-rate fp32 on the PE; ~11-bit mantissa
rounding of inputs, fp32 accumulation).
"""
import contextlib
import numpy as np

import concourse.bass as bass
import concourse.tile as tile
from concourse import mybir
from concourse.bass_utils import run_bass_kernel_spmd

F32 = mybir.dt.float32
F32R = mybir.dt.float32r
AF = mybir.ActivationFunctionType
OP = mybir.AluOpType

B, C = 4, 256
_L = 4096          # H*W; dev scripts may override before first use
_IBLK = 512        # query-block width (free dim of transposed score tiles)
EPS = 1e-5
NCORES = 8

_cache = {}


def _split_waits(nc, cap_ctrl=1, cap=1):
    """walrus in this container rejects >cap sync waits per instruction
    (CTRL-encoded ops: >cap_ctrl).  Move excess waits onto preceding NoOps."""
    ctrl = ("InstDrain", "InstNoOp", "InstEventSemaphore", "InstDmaTrigger")
    for fn in nc.m.functions:
        for bb in fn.blocks:
            insts = list(bb.instructions)
            out = []
            changed = False
            for inst in insts:
                si = inst.sync_info
                c = cap_ctrl if type(inst).__name__ in ctrl else cap
                if si is not None and len(si.on_wait) > c:
                    waits = list(si.on_wait)
                    extra, keep = waits[:-c], waits[-c:]
                    for k in range(0, len(extra), cap_ctrl):
                        nop = mybir.InstNoOp(
                            name=nc.get_next_instruction_name(), ins=[], outs=[])
                        nop.engine = inst.engine
                        nop.sync_info = mybir.SyncInfo(
                            on_wait=extra[k:k + cap_ctrl], on_update=[])
                        out.append(nop)
                        changed = True
                    inst.sync_info = mybir.SyncInfo(
                        on_wait=keep, on_update=list(si.on_update))
                out.append(inst)
            if changed:
                bb.instructions = out


def _build(L, IBLK):
    NI = L // IBLK
    NJ = L // 128
    NCH = max(1, L // 512)   # bn_stats chunks per partition row

    nc = bass.Bass(target_bir_lowering=False)

    d_in = {}
    def din(name, shape, dt=F32R):
        d_in[name] = nc.dram_tensor(name, list(shape), dt, kind="ExternalInput")
        return d_in[name]

    x_d = [din(f"x{t}", (128, L)) for t in range(2)]
    wq_d = [din(f"wq{t}", (128, 128)) for t in range(2)]
    wk_d = [din(f"wk{t}", (128, 128)) for t in range(2)]
    wv_d = [din(f"wv{t}", (128, 128)) for t in range(2)]
    wpA_d = din("wpA", (64, 256))
    wpB_d = din("wpB", (64, 256))
    bq_d = din("bq", (128, 1), F32)
    bk_d = din("bk", (128, 1), F32)
    bvA_d = din("bvA", (64, 1), F32)
    bvB_d = din("bvB", (64, 1), F32)
    gnw_d = [din(f"gnw{t}", (128, 1), F32) for t in range(2)]
    gnb_d = [din(f"gnb{t}", (128, 1), F32) for t in range(2)]
    gsel_d = din("gsel", (128, 16))
    gbc_d = din("gbc", (16, 128))
    ident_d = din("ident", (128, 64))
    ones_d = din("ones_in", (128, 64))
    part_d = nc.dram_tensor("part", [256, L], F32, kind="ExternalOutput")

    with tile.TileContext(nc) as tc, contextlib.ExitStack() as ctx:
        sing = ctx.enter_context(tc.tile_pool(name="sing", bufs=1))
        work = ctx.enter_context(tc.tile_pool(name="work", bufs=1))
        ps = ctx.enter_context(tc.tile_pool(name="ps", bufs=1, space="PSUM"))

        def stile(shape, dt, name, pool=sing, bufs=1, tag=None):
            return pool.tile(list(shape), dt, name=name, tag=tag or name,
                             bufs=bufs)

        # ---- load inputs ----
        x_sb = [stile((128, L), F32R, f"x{t}") for t in range(2)]
        wq_sb = [stile((128, 128), F32R, f"wq{t}") for t in range(2)]
        wk_sb = [stile((128, 128), F32R, f"wk{t}") for t in range(2)]
        wv_sb = [stile((128, 128), F32R, f"wv{t}") for t in range(2)]
        wpA = stile((64, 256), F32R, "wpA")
        wpB = stile((64, 256), F32R, "wpB")
        bq_sb = stile((128, 1), F32, "bq")
        bk_sb = stile((128, 1), F32, "bk")
        bvA_sb = stile((64, 1), F32, "bvA")
        bvB_sb = stile((64, 1), F32, "bvB")
        gnw_sb = [stile((128, 1), F32, f"gnw{t}") for t in range(2)]
        gnb_sb = [stile((128, 1), F32, f"gnb{t}") for t in range(2)]
        gsel = stile((128, 16), F32R, "gsel")
        gbc = stile((16, 128), F32R, "gbc")
        ident = stile((128, 64), F32R, "ident")
        ones = stile((128, 64), F32R, "ones_sb")
        for t in range(2):
            nc.sync.dma_start(out=x_sb[t][:], in_=x_d[t][:])
            nc.sync.dma_start(out=wq_sb[t][:], in_=wq_d[t][:])
            nc.sync.dma_start(out=wk_sb[t][:], in_=wk_d[t][:])
            nc.sync.dma_start(out=wv_sb[t][:], in_=wv_d[t][:])
            nc.sync.dma_start(out=gnw_sb[t][:], in_=gnw_d[t][:])
            nc.sync.dma_start(out=gnb_sb[t][:], in_=gnb_d[t][:])
        nc.sync.dma_start(out=wpA[:], in_=wpA_d[:])
        nc.sync.dma_start(out=wpB[:], in_=wpB_d[:])
        nc.sync.dma_start(out=bq_sb[:], in_=bq_d[:])
        nc.sync.dma_start(out=bk_sb[:], in_=bk_d[:])
        nc.sync.dma_start(out=bvA_sb[:], in_=bvA_d[:])
        nc.sync.dma_start(out=bvB_sb[:], in_=bvB_d[:])
        nc.sync.dma_start(out=gsel[:], in_=gsel_d[:])
        nc.sync.dma_start(out=gbc[:], in_=gbc_d[:])
        nc.sync.dma_start(out=ident[:], in_=ident_d[:])
        nc.sync.dma_start(out=ones[:], in_=ones_d[:])

        eps_t = stile((128, 1), F32, "eps_t")
        nc.vector.memset(eps_t[:], EPS)

        # ---- GroupNorm stats -> per-channel scale s_t / shift tb_t ----
        s_t = []
        tb_t = []
        for t in range(2):
            sta = stile((128, NCH, 6), F32, f"sta{t}", pool=work)
            for chnk in range(NCH):
                nc.vector.bn_stats(
                    out=sta[:, chnk, :],
                    in_=x_sb[t][:, 512 * chnk:512 * (chnk + 1)].bitcast(F32))
            mv = stile((128, 2), F32, f"mv{t}", pool=work)
            nc.vector.bn_aggr(out=mv[:], in_=sta[:])
            # stats2 = [mean, E[x^2]] per channel
            stats2 = stile((128, 2), F32, f"stats2_{t}", pool=work)
            nc.vector.tensor_copy(out=stats2[:, 0:1], in_=mv[:, 0:1])
            nc.vector.scalar_tensor_tensor(
                out=stats2[:, 1:2], in0=mv[:, 0:1], scalar=mv[:, 0:1],
                in1=mv[:, 1:2], op0=OP.mult, op1=OP.add)
            # group sums (16 groups per K-tile)
            psg = ps.tile([16, 2], F32, name=f"psg{t}", tag="sc", bufs=4)
            nc.tensor.matmul(psg[:], gsel[:].bitcast(F32), stats2[:], start=True, stop=True)
            gstats = stile((16, 2), F32, f"gstats{t}", pool=work)
            nc.vector.tensor_scalar_mul(
                out=gstats[:, 0:1], in0=psg[:, 0:1], scalar1=0.125)
            gm2 = stile((16, 1), F32, f"gm2_{t}", pool=work)
            nc.vector.tensor_mul(
                out=gm2[:], in0=gstats[:, 0:1], in1=gstats[:, 0:1])
            gvar = stile((16, 1), F32, f"gvar{t}", pool=work)
            nc.vector.scalar_tensor_tensor(
                out=gvar[:], in0=psg[:, 1:2], scalar=0.125, in1=gm2[:],
                op0=OP.mult, op1=OP.subtract)
            gsd = stile((16, 1), F32, f"gsd{t}", pool=work)
            nc.scalar.activation(out=gsd[:], in_=gvar[:], func=AF.Sqrt,
                                 bias=eps_t[0:16, :], scale=1.0)
            nc.vector.reciprocal(out=gstats[:, 1:2], in_=gsd[:])
            # broadcast groups -> channels
            psb = ps.tile([128, 2], F32, name=f"psb{t}", tag="sc", bufs=4)
            nc.tensor.matmul(psb[:], gbc[:].bitcast(F32), gstats[:], start=True, stop=True)
            s = stile((128, 1), F32R, f"s{t}", pool=work)
            nc.vector.tensor_mul(out=s[:], in0=psb[:, 1:2], in1=gnw_sb[t][:])
            ms = stile((128, 1), F32, f"ms{t}", pool=work)
            nc.vector.tensor_scalar_mul(
                out=ms[:], in0=psb[:, 0:1], scalar1=s[:].bitcast(F32))
            tb = stile((128, 1), F32, f"tb{t}", pool=work)
            nc.vector.tensor_sub(out=tb[:], in0=gnb_sb[t][:], in1=ms[:])
            s_t.append(s)
            tb_t.append(tb)

        # ---- fold GN scale into qkv weights; GN shift into biases ----
        wqs, wks, wvs = [], [], []
        for t in range(2):
            for (lbl, w_raw, lst) in (("q", wq_sb, wqs), ("k", wk_sb, wks),
                                      ("v", wv_sb, wvs)):
                ws = stile((128, 128), F32R, f"ws_{lbl}{t}", pool=work)
                nc.vector.tensor_scalar_mul(
                    out=ws[:], in0=w_raw[t][:].bitcast(F32),
                    scalar1=s_t[t][:].bitcast(F32))
                lst.append(ws)

        # bias_q = bq + sum_c wqT[c,o] * tb_c   (and same for k)
        bias_q = stile((128, 1), F32, "bias_q")
        bias_k = stile((128, 1), F32, "bias_k")
        bias_vA = stile((64, 1), F32, "bias_vA")
        bias_vB = stile((64, 1), F32, "bias_vB")
        for (w_raw, host_b, out_b) in ((wq_sb, bq_sb, bias_q),
                                       (wk_sb, bk_sb, bias_k)):
            pbias = ps.tile([128, 1], F32, name="pbias", tag="sc", bufs=4)
            nc.tensor.matmul(pbias[:], w_raw[0][:].bitcast(F32), tb_t[0][:],
                             start=True, stop=False)
            nc.tensor.matmul(pbias[:], w_raw[1][:].bitcast(F32), tb_t[1][:],
                             start=False, stop=True)
            nc.vector.tensor_add(out=out_b[:], in0=pbias[:], in1=host_b[:])
        for (cols, host_b, out_b) in ((slice(0, 64), bvA_sb, bias_vA),
                                      (slice(64, 128), bvB_sb, bias_vB)):
            pbias = ps.tile([64, 1], F32, name="pbiasv", tag="sc", bufs=4)
            nc.tensor.matmul(pbias[:], wv_sb[0][:, cols].bitcast(F32), tb_t[0][:],
                             start=True, stop=False)
            nc.tensor.matmul(pbias[:], wv_sb[1][:, cols].bitcast(F32), tb_t[1][:],
                             start=False, stop=True)
            nc.vector.tensor_add(out=out_b[:], in0=pbias[:], in1=host_b[:])

        # ---- qkv projection ----
        q_sb = stile((128, L), F32R, "q_sb")
        k_sb = stile((128, L), F32R, "k_sb")
        v_sb = stile((128, L), F32R, "v_sb")
        for n in range(L // 512):
            nsl = slice(512 * n, 512 * (n + 1))
            for (wlist, dst, bias) in ((wqs, q_sb, bias_q), (wks, k_sb, bias_k),
                                       (wvs, v_sb, None)):
                pqkv = ps.tile([128, 512], F32, name="pqkv", tag="sc", bufs=4)
                nc.tensor.matmul(pqkv[:], wlist[0][:], x_sb[0][:, nsl],
                                 start=True, stop=False)
                nc.tensor.matmul(pqkv[:], wlist[1][:], x_sb[1][:, nsl],
                                 start=False, stop=True)
                if bias is None:
                    nc.vector.tensor_copy(out=dst[:, nsl], in_=pqkv[:])
                else:
                    nc.vector.tensor_scalar_add(
                        out=dst[:, nsl], in0=pqkv[:], scalar1=bias[:])

        # ---- v transposes: vT[h] (128, NJ, 65); col 64 of each j-tile = 1 ----
        vT = {h: stile((128, NJ, 65), F32R, f"vT{h}") for h in (0, 1)}
        for h in (0, 1):
            nc.sync.dma_start(
                out=vT[h][:, :, 64:65],
                in_=ones_d[:, 0:NJ].rearrange("p (j o) -> p j o", o=1))
        for j in range(NJ):
            for h in (0, 1):
                pt = ps.tile([128, 64], F32R, name="pt", tag="sc", bufs=4)
                nc.tensor.transpose(
                    pt[:],
                    v_sb[64 * h:64 * h + 64, 128 * j:128 * (j + 1)],
                    ident[64 * h:64 * h + 64, 0:64])
                with nc.allow_low_precision(reason="f32r store"):
                    nc.vector.tensor_copy(out=vT[h][:, j, 0:64],
                                          in_=pt[:].bitcast(F32))

        # ---- attention + proj ----
        oT = {h: stile((64, L), F32R, f"oT{h}") for h in (0, 1)}
        out_sb = [stile((128, L), F32, f"out_sb{m}") for m in range(2)]

        for i in range(NI):
            isl = slice(IBLK * i, IBLK * (i + 1))
            po = {h: ps.tile([65, IBLK], F32, name=f"oacc{h}", tag=f"oacc{h}",
                             bufs=1) for h in (0, 1)}
            for j in range(NJ):
                jsl = slice(128 * j, 128 * (j + 1))
                psc = {h: ps.tile([128, IBLK], F32, name=f"sc{h}", tag="sc",
                                  bufs=4) for h in (0, 1)}
                for h in (0, 1):
                    hsl = slice(64 * h, 64 * h + 64)
                    nc.tensor.matmul(
                        psc[h][:], k_sb[hsl, jsl], q_sb[hsl, isl],
                        start=True, stop=True, tile_position=(64 * h, 0))
                e = {h: work.tile([128, IBLK], F32R, name=f"e{h}", tag=f"e{h}",
                                  bufs=3) for h in (0, 1)}
                for h in (0, 1):
                    nc.scalar.activation(out=e[h][:], in_=psc[h][:],
                                         func=AF.Exp, scale=1.0)
                for h in (0, 1):
                    nc.tensor.matmul(
                        po[h][:], vT[h][:, j, :], e[h][:],
                        start=(j == 0), stop=(j == NJ - 1))
            # normalize + bias_v -> oT
            for h, bias_v in ((0, bias_vA), (1, bias_vB)):
                recip = work.tile([128, IBLK], F32R, name="recip", tag="recip",
                                  bufs=2)
                with nc.allow_low_precision(reason="f32r recip"):
                    nc.vector.reciprocal(out=recip[64:65, :],
                                         in_=po[h][64:65, :])
                prep = ps.tile([64, IBLK], F32, name="prep", tag="sc", bufs=4)
                nc.tensor.matmul(prep[:], ones[64:65, 0:64],
                                 recip[64:65, :], start=True, stop=True)
                rep_sb = work.tile([64, IBLK], F32, name="rep", tag="rep",
                                   bufs=2)
                nc.vector.tensor_copy(out=rep_sb[:], in_=prep[:])
                nc.vector.tensor_mul(out=oT[h][:, isl], in0=po[h][0:64, :],
                                     in1=rep_sb[:])
                nc.vector.tensor_scalar_add(
                    out=oT[h][:, isl], in0=oT[h][:, isl].bitcast(F32),
                    scalar1=bias_v[:])
            # proj for this i-block
            for m in range(2):
                msl = slice(128 * m, 128 * (m + 1))
                pp = ps.tile([128, IBLK], F32, name="pp", tag="sc", bufs=4)
                nc.tensor.matmul(pp[:], wpA[:, msl], oT[0][:, isl],
                                 start=True, stop=False)
                nc.tensor.matmul(pp[:], wpB[:, msl], oT[1][:, isl],
                                 start=False, stop=True)
                nc.vector.tensor_copy(out=out_sb[m][:, isl], in_=pp[:])
                nc.sync.dma_start(out=part_d[msl, isl], in_=out_sb[m][:, isl])

    _split_waits(nc)
    return nc


def _host_prep(inputs, L):
    x = np.asarray(inputs["x"], dtype=np.float32)
    gn_w = np.asarray(inputs["gn_w"], dtype=np.float32)
    gn_b = np.asarray(inputs["gn_b"], dtype=np.float32)
    qkv_w = np.asarray(inputs["qkv_w"], dtype=np.float32)
    qkv_b = np.asarray(inputs["qkv_b"], dtype=np.float32)
    proj_w = np.asarray(inputs["proj_w"], dtype=np.float32)

    gsel = np.zeros((128, 16), np.float32)
    for cl in range(128):
        gsel[cl, cl // 8] = 1.0
    gbc = np.ascontiguousarray(gsel.T)
    ident = np.ascontiguousarray(np.tile(np.eye(64, dtype=np.float32), (2, 1)))
    ones = np.ones((128, 64), np.float32)

    in_maps = []
    for c in range(NCORES):
        b, hp = c // 2, c % 2
        xb = np.ascontiguousarray(x[b].reshape(C, L))
        slq = slice(128 * hp, 128 * (hp + 1))
        slk = slice(256 + 128 * hp, 256 + 128 * (hp + 1))
        slv = slice(512 + 128 * hp, 512 + 128 * (hp + 1))
        wqT = np.ascontiguousarray(qkv_w[slq].T) * 0.125
        wkT = np.ascontiguousarray(qkv_w[slk].T)
        wvT = np.ascontiguousarray(qkv_w[slv].T)
        wpT = np.ascontiguousarray(proj_w[:, 128 * hp:128 * (hp + 1)].T)
        bq = qkv_b[slq].reshape(128, 1) * 0.125
        bk = qkv_b[slk].reshape(128, 1)
        bv = qkv_b[slv].reshape(128, 1)
        m = {
            "x0": np.ascontiguousarray(xb[0:128]),
            "x1": np.ascontiguousarray(xb[128:256]),
            "wq0": np.ascontiguousarray(wqT[0:128]),
            "wq1": np.ascontiguousarray(wqT[128:256]),
            "wk0": np.ascontiguousarray(wkT[0:128]),
            "wk1": np.ascontiguousarray(wkT[128:256]),
            "wv0": np.ascontiguousarray(wvT[0:128]),
            "wv1": np.ascontiguousarray(wvT[128:256]),
            "wpA": np.ascontiguousarray(wpT[0:64]),
            "wpB": np.ascontiguousarray(wpT[64:128]),
            "bq": np.ascontiguousarray(bq),
            "bk": np.ascontiguousarray(bk),
            "bvA": np.ascontiguousarray(bv[0:64]),
            "bvB": np.ascontiguousarray(bv[64:128]),
            "gnw0": np.ascontiguousarray(gn_w[0:128].reshape(128, 1)),
            "gnw1": np.ascontiguousarray(gn_w[128:256].reshape(128, 1)),
            "gnb0": np.ascontiguousarray(gn_b[0:128].reshape(128, 1)),
            "gnb1": np.ascontiguousarray(gn_b[128:256].reshape(128, 1)),
            "gsel": gsel,
            "gbc": gbc,
            "ident": ident,
            "ones_in": ones,
        }
        in_maps.append(m)
    return in_maps


def _run(inputs, trace=False):
    L = _L
    key = (L, _IBLK)
    if key not in _cache:
        _cache[key] = _build(L, _IBLK)
    nc = _cache[key]
    in_maps = _host_prep(inputs, L)
    res = run_bass_kernel_spmd(nc, in_maps, core_ids=list(range(NCORES)),
                               trace=trace)
    x = np.asarray(inputs["x"], dtype=np.float32)
    proj_b = np.asarray(inputs["proj_b"], dtype=np.float32)
    out = np.empty((B, C, L), np.float32)
    for b in range(B):
        out[b] = (res.results[2 * b]["part"] + res.results[2 * b + 1]["part"]
                  + x[b].reshape(C, L) + proj_b[:, None])
    return out.reshape(B, C, x.shape[2], x.shape[3]).astype(np.float32), res


def kernel(**inputs):
    out, _ = _run(inputs, trace=False)
    return out


# revision 10
# speedup vs baseline: 1.4199x; 1.4199x over previous
"""AttentionBlock (GroupNorm + 4-head self-attention + proj + residual) on 8
Trainium2 NeuronCores.

Sharding: core c handles image b = c//2 and head-pair hp = c%2 (heads
2*hp, 2*hp+1, i.e. a contiguous 128-channel block of each of q/k/v).
Each core computes GroupNorm stats for its image, folds them into the qkv
weights, runs flash-style attention for its two heads (no max subtraction;
scores are ~N(0,1) so exp never overflows), projects through its 128-column
block of proj_w, and returns a partial (256, 4096) output.  Host sums the
two partials per image and adds the residual x and proj bias.

All matmuls run in float32r (full-rate fp32 on the PE; ~11-bit mantissa
rounding of inputs, fp32 accumulation).
"""
import contextlib
import numpy as np

import concourse.bass as bass
import concourse.tile as tile
from concourse import mybir
from concourse.bass_utils import run_bass_kernel_spmd

F32 = mybir.dt.float32
F32R = mybir.dt.float32r
AF = mybir.ActivationFunctionType
OP = mybir.AluOpType

B, C = 4, 256
_L = 4096          # H*W; dev scripts may override before first use
_IBLK = 1024       # query-block width (free dim of transposed score tiles)
EPS = 1e-5
NCORES = 8

_cache = {}


def _split_waits(nc, cap_ctrl=1, cap=1):
    """walrus in this container rejects >cap sync waits per instruction
    (CTRL-encoded ops: >cap_ctrl).  Move excess waits onto preceding NoOps."""
    ctrl = ("InstDrain", "InstNoOp", "InstEventSemaphore", "InstDmaTrigger")
    for fn in nc.m.functions:
        for bb in fn.blocks:
            insts = list(bb.instructions)
            out = []
            changed = False
            for inst in insts:
                si = inst.sync_info
                c = cap_ctrl if type(inst).__name__ in ctrl else cap
                if si is not None and len(si.on_wait) > c:
                    waits = list(si.on_wait)
                    extra, keep = waits[:-c], waits[-c:]
                    for k in range(0, len(extra), cap_ctrl):
                        nop = mybir.InstNoOp(
                            name=nc.get_next_instruction_name(), ins=[], outs=[])
                        nop.engine = inst.engine
                        nop.sync_info = mybir.SyncInfo(
                            on_wait=extra[k:k + cap_ctrl], on_update=[])
                        out.append(nop)
                        changed = True
                    inst.sync_info = mybir.SyncInfo(
                        on_wait=keep, on_update=list(si.on_update))
                out.append(inst)
            if changed:
                bb.instructions = out


def _build(L, IBLK):
    NI = L // IBLK
    NJ = L // 128
    NCH = max(1, L // 512)   # bn_stats chunks per partition row

    nc = bass.Bass(target_bir_lowering=False)

    d_in = {}
    def din(name, shape, dt=F32R):
        d_in[name] = nc.dram_tensor(name, list(shape), dt, kind="ExternalInput")
        return d_in[name]

    x_d = [din(f"x{t}", (128, L)) for t in range(2)]
    wq_d = [din(f"wq{t}", (128, 128)) for t in range(2)]
    wk_d = [din(f"wk{t}", (128, 128)) for t in range(2)]
    wv_d = [din(f"wv{t}", (128, 128)) for t in range(2)]
    wpA_d = din("wpA", (64, 256))
    wpB_d = din("wpB", (64, 256))
    bq_d = din("bq", (128, 1), F32)
    bk_d = din("bk", (128, 1), F32)
    bvA_d = din("bvA", (64, 1), F32)
    bvB_d = din("bvB", (64, 1), F32)
    gnw_d = [din(f"gnw{t}", (128, 1), F32) for t in range(2)]
    gnb_d = [din(f"gnb{t}", (128, 1), F32) for t in range(2)]
    gsel_d = din("gsel", (128, 16))
    gbc_d = din("gbc", (16, 128))
    ident_d = din("ident", (128, 64))
    ones_d = din("ones_in", (128, 64))
    zeros_d = din("zeros_in", (1, L))
    part_d = nc.dram_tensor("part", [256, L], F32R, kind="ExternalOutput")

    with tile.TileContext(nc) as tc, contextlib.ExitStack() as ctx:
        sing = ctx.enter_context(tc.tile_pool(name="sing", bufs=1))
        work = ctx.enter_context(tc.tile_pool(name="work", bufs=1))
        ps = ctx.enter_context(tc.tile_pool(name="ps", bufs=1, space="PSUM"))

        def stile(shape, dt, name, pool=sing, bufs=1, tag=None):
            return pool.tile(list(shape), dt, name=name, tag=tag or name,
                             bufs=bufs)

        _scctr = [0]
        def pstile(shape, name):
            """Transient PSUM tile; alternates between the two score slots."""
            _scctr[0] += 1
            tag = "scA" if _scctr[0] % 2 else "scB"
            return ps.tile(list(shape), F32, name=name, tag=tag, bufs=1)

        # ---- load inputs ----
        x_sb = [stile((128, L), F32R, f"x{t}") for t in range(2)]
        wq_sb = [stile((128, 128), F32R, f"wq{t}") for t in range(2)]
        wk_sb = [stile((128, 128), F32R, f"wk{t}") for t in range(2)]
        wv_sb = [stile((128, 128), F32R, f"wv{t}") for t in range(2)]
        wpA = stile((64, 256), F32R, "wpA")
        wpB = stile((64, 256), F32R, "wpB")
        bq_sb = stile((128, 1), F32, "bq")
        bk_sb = stile((128, 1), F32, "bk")
        bvA_sb = stile((64, 1), F32, "bvA")
        bvB_sb = stile((64, 1), F32, "bvB")
        gnw_sb = [stile((128, 1), F32, f"gnw{t}") for t in range(2)]
        gnb_sb = [stile((128, 1), F32, f"gnb{t}") for t in range(2)]
        gsel = stile((128, 16), F32R, "gsel")
        gbc = stile((16, 128), F32R, "gbc")
        ident = stile((128, 64), F32R, "ident")
        ones = stile((128, 64), F32R, "ones_sb")
        for t in range(2):
            nc.sync.dma_start(out=x_sb[t][:], in_=x_d[t][:])
            nc.sync.dma_start(out=wq_sb[t][:], in_=wq_d[t][:])
            nc.sync.dma_start(out=wk_sb[t][:], in_=wk_d[t][:])
            nc.sync.dma_start(out=wv_sb[t][:], in_=wv_d[t][:])
            nc.sync.dma_start(out=gnw_sb[t][:], in_=gnw_d[t][:])
            nc.sync.dma_start(out=gnb_sb[t][:], in_=gnb_d[t][:])
        nc.sync.dma_start(out=wpA[:], in_=wpA_d[:])
        nc.sync.dma_start(out=wpB[:], in_=wpB_d[:])
        nc.sync.dma_start(out=bq_sb[:], in_=bq_d[:])
        nc.sync.dma_start(out=bk_sb[:], in_=bk_d[:])
        nc.sync.dma_start(out=bvA_sb[:], in_=bvA_d[:])
        nc.sync.dma_start(out=bvB_sb[:], in_=bvB_d[:])
        nc.sync.dma_start(out=gsel[:], in_=gsel_d[:])
        nc.sync.dma_start(out=gbc[:], in_=gbc_d[:])
        nc.sync.dma_start(out=ident[:], in_=ident_d[:])
        nc.sync.dma_start(out=ones[:], in_=ones_d[:])

        eps_t = stile((128, 1), F32, "eps_t")
        nc.vector.memset(eps_t[:], EPS)

        # ---- GroupNorm stats -> per-channel scale s_t / shift tb_t ----
        s_t = []
        tb_t = []
        for t in range(2):
            sta = stile((128, NCH, 6), F32, f"sta{t}", pool=work)
            for chnk in range(NCH):
                nc.vector.bn_stats(
                    out=sta[:, chnk, :],
                    in_=x_sb[t][:, 512 * chnk:512 * (chnk + 1)].bitcast(F32))
            mv = stile((128, 2), F32, f"mv{t}", pool=work)
            nc.vector.bn_aggr(out=mv[:], in_=sta[:])
            # stats2 = [mean, E[x^2]] per channel
            stats2 = stile((128, 2), F32, f"stats2_{t}", pool=work)
            nc.vector.tensor_copy(out=stats2[:, 0:1], in_=mv[:, 0:1])
            nc.vector.scalar_tensor_tensor(
                out=stats2[:, 1:2], in0=mv[:, 0:1], scalar=mv[:, 0:1],
                in1=mv[:, 1:2], op0=OP.mult, op1=OP.add)
            # group sums (16 groups per K-tile)
            psg = pstile((16, 2), f"psg{t}")
            nc.tensor.matmul(psg[:], gsel[:].bitcast(F32), stats2[:], start=True, stop=True)
            gstats = stile((16, 2), F32, f"gstats{t}", pool=work)
            nc.vector.tensor_scalar_mul(
                out=gstats[:, 0:1], in0=psg[:, 0:1], scalar1=0.125)
            gm2 = stile((16, 1), F32, f"gm2_{t}", pool=work)
            nc.vector.tensor_mul(
                out=gm2[:], in0=gstats[:, 0:1], in1=gstats[:, 0:1])
            gvar = stile((16, 1), F32, f"gvar{t}", pool=work)
            nc.vector.scalar_tensor_tensor(
                out=gvar[:], in0=psg[:, 1:2], scalar=0.125, in1=gm2[:],
                op0=OP.mult, op1=OP.subtract)
            gsd = stile((16, 1), F32, f"gsd{t}", pool=work)
            nc.scalar.activation(out=gsd[:], in_=gvar[:], func=AF.Sqrt,
                                 bias=eps_t[0:16, :], scale=1.0)
            nc.vector.reciprocal(out=gstats[:, 1:2], in_=gsd[:])
            # broadcast groups -> channels
            psb = pstile((128, 2), f"psb{t}")
            nc.tensor.matmul(psb[:], gbc[:].bitcast(F32), gstats[:], start=True, stop=True)
            s = stile((128, 1), F32R, f"s{t}", pool=work)
            nc.vector.tensor_mul(out=s[:], in0=psb[:, 1:2], in1=gnw_sb[t][:])
            ms = stile((128, 1), F32, f"ms{t}", pool=work)
            nc.vector.tensor_scalar_mul(
                out=ms[:], in0=psb[:, 0:1], scalar1=s[:].bitcast(F32))
            tb = stile((128, 1), F32, f"tb{t}", pool=work)
            nc.vector.tensor_sub(out=tb[:], in0=gnb_sb[t][:], in1=ms[:])
            s_t.append(s)
            tb_t.append(tb)

        # ---- fold GN scale into qkv weights; GN shift into biases ----
        wqs, wks, wvs = [], [], []
        for t in range(2):
            for (lbl, w_raw, lst) in (("q", wq_sb, wqs), ("k", wk_sb, wks),
                                      ("v", wv_sb, wvs)):
                ws = stile((128, 128), F32R, f"ws_{lbl}{t}", pool=work)
                nc.vector.tensor_scalar_mul(
                    out=ws[:], in0=w_raw[t][:].bitcast(F32),
                    scalar1=s_t[t][:].bitcast(F32))
                lst.append(ws)

        # bias_q = bq + sum_c wqT[c,o] * tb_c   (and same for k)
        bias_q = stile((128, 1), F32, "bias_q")
        bias_k = stile((128, 1), F32, "bias_k")
        bias_vA = stile((64, 1), F32, "bias_vA")
        bias_vB = stile((64, 1), F32, "bias_vB")
        for (w_raw, host_b, out_b) in ((wq_sb, bq_sb, bias_q),
                                       (wk_sb, bk_sb, bias_k)):
            pbias = pstile((128, 1), "pbias")
            nc.tensor.matmul(pbias[:], w_raw[0][:].bitcast(F32), tb_t[0][:],
                             start=True, stop=False)
            nc.tensor.matmul(pbias[:], w_raw[1][:].bitcast(F32), tb_t[1][:],
                             start=False, stop=True)
            nc.vector.tensor_add(out=out_b[:], in0=pbias[:], in1=host_b[:])
        for (cols, host_b, out_b) in ((slice(0, 64), bvA_sb, bias_vA),
                                      (slice(64, 128), bvB_sb, bias_vB)):
            pbias = pstile((64, 1), "pbiasv")
            nc.tensor.matmul(pbias[:], wv_sb[0][:, cols].bitcast(F32), tb_t[0][:],
                             start=True, stop=False)
            nc.tensor.matmul(pbias[:], wv_sb[1][:, cols].bitcast(F32), tb_t[1][:],
                             start=False, stop=True)
            nc.vector.tensor_add(out=out_b[:], in0=pbias[:], in1=host_b[:])

        # ---- qkv projection ----
        # k is stored per-head, zero-padded to K=128 so the scores matmuls
        # run with the full contraction dim (K=64 matmuls never un-throttle
        # the PE HAM clock gate).
        q_sb = stile((128, L), F32R, "q_sb")
        kp = {h: stile((128, L), F32R, f"kp{h}") for h in (0, 1)}
        v_sb = stile((128, L), F32R, "v_sb")
        zsrc = zeros_d[:]
        for h in (0, 1):
            zpad = bass.AP(tensor=zsrc.tensor, offset=zsrc.offset,
                           ap=[[0, 64]] + list(zsrc.ap)[1:])
            dst = kp[h][64:128, :] if h == 0 else kp[h][0:64, :]
            nc.sync.dma_start(out=dst, in_=zpad)
        for n in range(L // 512):
            nsl = slice(512 * n, 512 * (n + 1))
            for (wlist, kind) in ((wqs, "q"), (wks, "k"), (wvs, "v")):
                pqkv = pstile((128, 512), f"pqkv{kind}")
                nc.tensor.matmul(pqkv[:], wlist[0][:], x_sb[0][:, nsl],
                                 start=True, stop=False)
                nc.tensor.matmul(pqkv[:], wlist[1][:], x_sb[1][:, nsl],
                                 start=False, stop=True)
                if kind == "q":
                    nc.vector.tensor_scalar_add(
                        out=q_sb[:, nsl], in0=pqkv[:], scalar1=bias_q[:])
                elif kind == "v":
                    nc.vector.tensor_copy(out=v_sb[:, nsl], in_=pqkv[:])
                else:
                    nc.vector.tensor_scalar_add(
                        out=kp[0][0:64, nsl], in0=pqkv[0:64, :],
                        scalar1=bias_k[0:64, :])
                    nc.vector.tensor_scalar_add(
                        out=kp[1][64:128, nsl], in0=pqkv[64:128, :],
                        scalar1=bias_k[64:128, :])

        # ---- v transposes: vT[h] (128, NJ, 65); col 64 of each j-tile = 1 ----
        vT = {h: stile((128, NJ, 65), F32R, f"vT{h}") for h in (0, 1)}
        for h in (0, 1):
            nc.sync.dma_start(
                out=vT[h][:, :, 64:65],
                in_=ones_d[:, 0:NJ].rearrange("p (j o) -> p j o", o=1))
        for j in range(NJ):
            for h in (0, 1):
                _scctr[0] += 1
                pt = ps.tile([128, 64], F32R, name="pt",
                             tag="scA" if _scctr[0] % 2 else "scB", bufs=1)
                nc.tensor.transpose(
                    pt[:],
                    v_sb[64 * h:64 * h + 64, 128 * j:128 * (j + 1)],
                    ident[64 * h:64 * h + 64, 0:64])
                with nc.allow_low_precision(reason="f32r store"):
                    nc.vector.tensor_copy(out=vT[h][:, j, 0:64],
                                          in_=pt[:].bitcast(F32))

        # ---- attention + proj ----
        # x tiles are dead after the qkv matmuls; reuse them (bitcast to f32)
        # as the proj output staging buffers to stay inside SBUF.
        oT = {h: stile((64, L), F32R, f"oT{h}") for h in (0, 1)}
        out_sb = [x_sb[m][:] for m in range(2)]

        for i in range(NI):
            isl = slice(IBLK * i, IBLK * (i + 1))
            po = {h: ps.tile([65, IBLK], F32, name=f"oacc{h}", tag=f"oacc{h}",
                             bufs=1) for h in (0, 1)}
            for j in range(NJ):
                jsl = slice(128 * j, 128 * (j + 1))
                psc = {h: ps.tile([128, IBLK], F32, name=f"sc{h}",
                                  tag=("scA", "scB")[h], bufs=1)
                       for h in (0, 1)}
                for h in (0, 1):
                    for u in range(IBLK // 512):
                        usl = slice(IBLK * i + 512 * u,
                                    IBLK * i + 512 * (u + 1))
                        nc.tensor.matmul(
                            psc[h][:, 512 * u:512 * (u + 1)],
                            kp[h][:, jsl], q_sb[:, usl],
                            start=True, stop=True)
                    nc.scalar.activation(
                        out=(e := work.tile([128, IBLK], F32R, name=f"e{h}",
                                            tag=f"e{h}", bufs=2))[:],
                        in_=psc[h][:], func=AF.Exp, scale=1.0)
                    for u in range(IBLK // 512):
                        nc.tensor.matmul(
                            po[h][:, 512 * u:512 * (u + 1)],
                            vT[h][:, j, :], e[:, 512 * u:512 * (u + 1)],
                            start=(j == 0), stop=(j == NJ - 1))
            # normalize + bias_v -> oT
            for h, bias_v in ((0, bias_vA), (1, bias_vB)):
                recip = work.tile([128, IBLK], F32, name="recip", tag="recip",
                                  bufs=2)
                nc.vector.reciprocal(out=recip[64:65, :],
                                     in_=po[h][64:65, :])
                prep = ps.tile([64, IBLK], F32, name="prep",
                               tag=("scA", "scB")[h], bufs=1)
                for u in range(IBLK // 512):
                    nc.tensor.matmul(
                        prep[:, 512 * u:512 * (u + 1)],
                        ones[64:65, 0:64].bitcast(F32),
                        recip[64:65, 512 * u:512 * (u + 1)],
                        start=True, stop=True)
                rep_sb = work.tile([64, IBLK], F32, name="rep", tag="rep",
                                   bufs=2)
                nc.vector.tensor_copy(out=rep_sb[:], in_=prep[:])
                nc.vector.tensor_mul(out=oT[h][:, isl], in0=po[h][0:64, :],
                                     in1=rep_sb[:])
                nc.vector.tensor_scalar_add(
                    out=oT[h][:, isl], in0=oT[h][:, isl].bitcast(F32),
                    scalar1=bias_v[:])
            # proj for this i-block
            for m in range(2):
                msl = slice(128 * m, 128 * (m + 1))
                pp = ps.tile([128, IBLK], F32, name="pp", tag=f"oacc{m}",
                             bufs=1)
                for u in range(IBLK // 512):
                    ul = slice(512 * u, 512 * (u + 1))
                    uabs = slice(IBLK * i + 512 * u, IBLK * i + 512 * (u + 1))
                    nc.tensor.matmul(pp[:, ul], wpA[:, msl], oT[0][:, uabs],
                                     start=True, stop=False)
                    nc.tensor.matmul(pp[:, ul], wpB[:, msl], oT[1][:, uabs],
                                     start=False, stop=True)
                with nc.allow_low_precision(reason="f32r out stage"):
                    nc.vector.tensor_copy(out=out_sb[m][:, isl], in_=pp[:])
                nc.sync.dma_start(out=part_d[msl, isl], in_=out_sb[m][:, isl])

    _split_waits(nc)
    return nc


def _host_prep(inputs, L):
    x = np.asarray(inputs["x"], dtype=np.float32)
    gn_w = np.asarray(inputs["gn_w"], dtype=np.float32)
    gn_b = np.asarray(inputs["gn_b"], dtype=np.float32)
    qkv_w = np.asarray(inputs["qkv_w"], dtype=np.float32)
    qkv_b = np.asarray(inputs["qkv_b"], dtype=np.float32)
    proj_w = np.asarray(inputs["proj_w"], dtype=np.float32)

    gsel = np.zeros((128, 16), np.float32)
    for cl in range(128):
        gsel[cl, cl // 8] = 1.0
    gbc = np.ascontiguousarray(gsel.T)
    ident = np.ascontiguousarray(np.tile(np.eye(64, dtype=np.float32), (2, 1)))
    ones = np.ones((128, 64), np.float32)
    zeros = np.zeros((1, L), np.float32)

    in_maps = []
    for c in range(NCORES):
        b, hp = c // 2, c % 2
        xb = np.ascontiguousarray(x[b].reshape(C, L))
        slq = slice(128 * hp, 128 * (hp + 1))
        slk = slice(256 + 128 * hp, 256 + 128 * (hp + 1))
        slv = slice(512 + 128 * hp, 512 + 128 * (hp + 1))
        wqT = np.ascontiguousarray(qkv_w[slq].T) * 0.125
        wkT = np.ascontiguousarray(qkv_w[slk].T)
        wvT = np.ascontiguousarray(qkv_w[slv].T)
        wpT = np.ascontiguousarray(proj_w[:, 128 * hp:128 * (hp + 1)].T)
        bq = qkv_b[slq].reshape(128, 1) * 0.125
        bk = qkv_b[slk].reshape(128, 1)
        bv = qkv_b[slv].reshape(128, 1)
        m = {
            "x0": np.ascontiguousarray(xb[0:128]),
            "x1": np.ascontiguousarray(xb[128:256]),
            "wq0": np.ascontiguousarray(wqT[0:128]),
            "wq1": np.ascontiguousarray(wqT[128:256]),
            "wk0": np.ascontiguousarray(wkT[0:128]),
            "wk1": np.ascontiguousarray(wkT[128:256]),
            "wv0": np.ascontiguousarray(wvT[0:128]),
            "wv1": np.ascontiguousarray(wvT[128:256]),
            "wpA": np.ascontiguousarray(wpT[0:64]),
            "wpB": np.ascontiguousarray(wpT[64:128]),
            "bq": np.ascontiguousarray(bq),
            "bk": np.ascontiguousarray(bk),
            "bvA": np.ascontiguousarray(bv[0:64]),
            "bvB": np.ascontiguousarray(bv[64:128]),
            "gnw0": np.ascontiguousarray(gn_w[0:128].reshape(128, 1)),
            "gnw1": np.ascontiguousarray(gn_w[128:256].reshape(128, 1)),
            "gnb0": np.ascontiguousarray(gn_b[0:128].reshape(128, 1)),
            "gnb1": np.ascontiguousarray(gn_b[128:256].reshape(128, 1)),
            "gsel": gsel,
            "gbc": gbc,
            "ident": ident,
            "ones_in": ones,
            "zeros_in": zeros,
        }
        in_maps.append(m)
    return in_maps


def _run(inputs, trace=False):
    L = _L
    key = (L, _IBLK)
    if key not in _cache:
        _cache[key] = _build(L, _IBLK)
    nc = _cache[key]
    in_maps = _host_prep(inputs, L)
    res = run_bass_kernel_spmd(nc, in_maps, core_ids=list(range(NCORES)),
                               trace=trace)
    x = np.asarray(inputs["x"], dtype=np.float32)
    proj_b = np.asarray(inputs["proj_b"], dtype=np.float32)
    out = np.empty((B, C, L), np.float32)
    for b in range(B):
        out[b] = (res.results[2 * b]["part"] + res.results[2 * b + 1]["part"]
                  + x[b].reshape(C, L) + proj_b[:, None])
    return out.reshape(B, C, x.shape[2], x.shape[3]).astype(np.float32), res


def kernel(**inputs):
    out, _ = _run(inputs, trace=False)
    return out


# revision 11
# speedup vs baseline: 1.7338x; 1.2211x over previous
"""AttentionBlock (GroupNorm + 4-head self-attention + proj + residual) on 8
Trainium2 NeuronCores.

Sharding: core c handles image b = c//2 and head-pair hp = c%2 (heads
2*hp, 2*hp+1, i.e. a contiguous 128-channel block of each of q/k/v).
Each core computes GroupNorm stats for its image (f32), folds them into the
qkv weights, runs flash-style attention for its two heads (no max
subtraction; scores are ~N(0,1) so exp never overflows), projects through
its 128-column block of proj_w, and returns a partial (256, 4096) output.
Host sums the two partials per image and adds the residual x and proj bias.

Matmuls run in bf16 (fp32 PSUM accumulation).  k is stored per-head
zero-padded to K=128 because K=64 matmuls never un-throttle the PE HAM
clock gate (measured: K=64 streams run at 1.2 GHz forever).
"""
import contextlib
import numpy as np

import concourse.bass as bass
import concourse.tile as tile
from concourse import mybir
from concourse.bass_utils import run_bass_kernel_spmd

F32 = mybir.dt.float32
BF16 = mybir.dt.bfloat16
AF = mybir.ActivationFunctionType
OP = mybir.AluOpType
NPBF16 = mybir.dt.np(mybir.dt.bfloat16)

B, C = 4, 256
_L = 4096          # H*W; dev scripts may override before first use
_IBLK = 1024       # query-block width (free dim of transposed score tiles)
EPS = 1e-5
NCORES = 8

_cache = {}


def _split_waits(nc, cap_ctrl=1, cap=1):
    """walrus in this container rejects >1 sync wait per instruction.
    Move excess waits onto preceding NoOps on the same engine."""
    for fn in nc.m.functions:
        for bb in fn.blocks:
            insts = list(bb.instructions)
            out = []
            changed = False
            for inst in insts:
                si = inst.sync_info
                c = cap
                if si is not None and len(si.on_wait) > c:
                    waits = list(si.on_wait)
                    extra, keep = waits[:-c], waits[-c:]
                    for k in range(0, len(extra), cap_ctrl):
                        nop = mybir.InstNoOp(
                            name=nc.get_next_instruction_name(), ins=[], outs=[])
                        nop.engine = inst.engine
                        nop.sync_info = mybir.SyncInfo(
                            on_wait=extra[k:k + cap_ctrl], on_update=[])
                        out.append(nop)
                        changed = True
                    inst.sync_info = mybir.SyncInfo(
                        on_wait=keep, on_update=list(si.on_update))
                out.append(inst)
            if changed:
                bb.instructions = out


def _build(L, IBLK):
    NI = L // IBLK
    NJ = L // 128
    NCH = max(1, L // 512)   # bn_stats chunks per partition row

    nc = bass.Bass(target_bir_lowering=False)

    def din(name, shape, dt=BF16):
        return nc.dram_tensor(name, list(shape), dt, kind="ExternalInput")

    x_d = [din(f"x{t}", (128, L)) for t in range(2)]
    xf_d = [din(f"xf{t}", (128, L), F32) for t in range(2)]
    wq_d = [din(f"wq{t}", (128, 128)) for t in range(2)]
    wk_d = [din(f"wk{t}", (128, 128)) for t in range(2)]
    wv_d = [din(f"wv{t}", (128, 128)) for t in range(2)]
    wpA_d = din("wpA", (64, 256))
    wpB_d = din("wpB", (64, 256))
    bq_d = din("bq", (128, 1), F32)
    bk_d = din("bk", (128, 1), F32)
    bvA_d = din("bvA", (64, 1), F32)
    bvB_d = din("bvB", (64, 1), F32)
    gnw_d = [din(f"gnw{t}", (128, 1), F32) for t in range(2)]
    gnb_d = [din(f"gnb{t}", (128, 1), F32) for t in range(2)]
    gsel_d = din("gsel", (128, 16), F32)
    gbc_d = din("gbc", (16, 128), F32)
    ones_d = din("ones_in", (128, 64))
    zeros_d = din("zeros_in", (1, L))
    part_d = nc.dram_tensor("part", [256, L], F32, kind="ExternalOutput")

    with tile.TileContext(nc) as tc, contextlib.ExitStack() as ctx:
        sing = ctx.enter_context(tc.tile_pool(name="sing", bufs=1))
        work = ctx.enter_context(tc.tile_pool(name="work", bufs=1))
        ps = ctx.enter_context(tc.tile_pool(name="ps", bufs=1, space="PSUM"))

        def stile(shape, dt, name, pool=sing, bufs=1, tag=None):
            return pool.tile(list(shape), dt, name=name, tag=tag or name,
                             bufs=bufs)

        _scctr = [0]
        def pstile(shape, name, dt=F32):
            """Transient PSUM tile; alternates between the two score slots."""
            _scctr[0] += 1
            tag = "scA" if _scctr[0] % 2 else "scB"
            return ps.tile(list(shape), dt, name=name, tag=tag, bufs=1)

        # ---- load inputs ----
        x_sb = [stile((128, L), BF16, f"x{t}") for t in range(2)]
        xf_sb = [stile((128, L), F32, f"xf{t}") for t in range(2)]
        wq_sb = [stile((128, 128), BF16, f"wq{t}") for t in range(2)]
        wk_sb = [stile((128, 128), BF16, f"wk{t}") for t in range(2)]
        wv_sb = [stile((128, 128), BF16, f"wv{t}") for t in range(2)]
        wpA = stile((64, 256), BF16, "wpA")
        wpB = stile((64, 256), BF16, "wpB")
        bq_sb = stile((128, 1), F32, "bq")
        bk_sb = stile((128, 1), F32, "bk")
        bvA_sb = stile((64, 1), F32, "bvA")
        bvB_sb = stile((64, 1), F32, "bvB")
        gnw_sb = [stile((128, 1), F32, f"gnw{t}") for t in range(2)]
        gnb_sb = [stile((128, 1), F32, f"gnb{t}") for t in range(2)]
        gsel = stile((128, 16), F32, "gsel")
        gbc = stile((16, 128), F32, "gbc")
        ones = stile((128, 64), BF16, "ones_sb")
        for t in range(2):
            nc.sync.dma_start(out=x_sb[t][:], in_=x_d[t][:])
            nc.sync.dma_start(out=xf_sb[t][:], in_=xf_d[t][:])
            nc.sync.dma_start(out=wq_sb[t][:], in_=wq_d[t][:])
            nc.sync.dma_start(out=wk_sb[t][:], in_=wk_d[t][:])
            nc.sync.dma_start(out=wv_sb[t][:], in_=wv_d[t][:])
            nc.sync.dma_start(out=gnw_sb[t][:], in_=gnw_d[t][:])
            nc.sync.dma_start(out=gnb_sb[t][:], in_=gnb_d[t][:])
        nc.sync.dma_start(out=wpA[:], in_=wpA_d[:])
        nc.sync.dma_start(out=wpB[:], in_=wpB_d[:])
        nc.sync.dma_start(out=bq_sb[:], in_=bq_d[:])
        nc.sync.dma_start(out=bk_sb[:], in_=bk_d[:])
        nc.sync.dma_start(out=bvA_sb[:], in_=bvA_d[:])
        nc.sync.dma_start(out=bvB_sb[:], in_=bvB_d[:])
        nc.sync.dma_start(out=gsel[:], in_=gsel_d[:])
        nc.sync.dma_start(out=gbc[:], in_=gbc_d[:])
        nc.sync.dma_start(out=ones[:], in_=ones_d[:])

        eps_t = stile((128, 1), F32, "eps_t")
        nc.vector.memset(eps_t[:], EPS)

        # ---- GroupNorm stats (f32 x copy) -> per-channel scale/shift ----
        s_t, tb_t = [], []
        for t in range(2):
            sta = stile((128, NCH, 6), F32, f"sta{t}", pool=work)
            for chnk in range(NCH):
                nc.vector.bn_stats(
                    out=sta[:, chnk, :],
                    in_=xf_sb[t][:, 512 * chnk:512 * (chnk + 1)])
            mv = stile((128, 2), F32, f"mv{t}", pool=work)
            nc.vector.bn_aggr(out=mv[:], in_=sta[:])
            stats2 = stile((128, 2), F32, f"stats2_{t}", pool=work)
            nc.vector.tensor_copy(out=stats2[:, 0:1], in_=mv[:, 0:1])
            nc.vector.scalar_tensor_tensor(
                out=stats2[:, 1:2], in0=mv[:, 0:1], scalar=mv[:, 0:1],
                in1=mv[:, 1:2], op0=OP.mult, op1=OP.add)
            psg = pstile((16, 2), f"psg{t}")
            nc.tensor.matmul(psg[:], gsel[:], stats2[:], start=True, stop=True)
            gstats = stile((16, 2), F32, f"gstats{t}", pool=work)
            nc.vector.tensor_scalar_mul(
                out=gstats[:, 0:1], in0=psg[:, 0:1], scalar1=0.125)
            gm2 = stile((16, 1), F32, f"gm2_{t}", pool=work)
            nc.vector.tensor_mul(
                out=gm2[:], in0=gstats[:, 0:1], in1=gstats[:, 0:1])
            gvar = stile((16, 1), F32, f"gvar{t}", pool=work)
            nc.vector.scalar_tensor_tensor(
                out=gvar[:], in0=psg[:, 1:2], scalar=0.125, in1=gm2[:],
                op0=OP.mult, op1=OP.subtract)
            gsd = stile((16, 1), F32, f"gsd{t}", pool=work)
            nc.scalar.activation(out=gsd[:], in_=gvar[:], func=AF.Sqrt,
                                 bias=eps_t[0:16, :], scale=1.0)
            nc.vector.reciprocal(out=gstats[:, 1:2], in_=gsd[:])
            psb = pstile((128, 2), f"psb{t}")
            nc.tensor.matmul(psb[:], gbc[:], gstats[:], start=True, stop=True)
            s = stile((128, 1), F32, f"s{t}", pool=work)
            nc.vector.tensor_mul(out=s[:], in0=psb[:, 1:2], in1=gnw_sb[t][:])
            ms = stile((128, 1), F32, f"ms{t}", pool=work)
            nc.vector.tensor_scalar_mul(
                out=ms[:], in0=psb[:, 0:1], scalar1=s[:])
            tb = stile((128, 1), BF16, f"tb{t}", pool=work)
            with nc.allow_low_precision(reason="bf16 shift"):
                nc.vector.tensor_sub(out=tb[:], in0=gnb_sb[t][:], in1=ms[:])
            s_t.append(s)
            tb_t.append(tb)

        # ---- fold GN scale into qkv weights; GN shift into biases ----
        wqs, wks, wvs = [], [], []
        for t in range(2):
            for (lbl, w_raw, lst) in (("q", wq_sb, wqs), ("k", wk_sb, wks),
                                      ("v", wv_sb, wvs)):
                ws = stile((128, 128), BF16, f"ws_{lbl}{t}", pool=work)
                nc.vector.tensor_scalar_mul(
                    out=ws[:], in0=w_raw[t][:], scalar1=s_t[t][:])
                lst.append(ws)

        bias_q = stile((128, 1), F32, "bias_q")
        bias_k = stile((128, 1), F32, "bias_k")
        bias_vA = stile((64, 1), F32, "bias_vA")
        bias_vB = stile((64, 1), F32, "bias_vB")
        for (w_raw, host_b, out_b) in ((wq_sb, bq_sb, bias_q),
                                       (wk_sb, bk_sb, bias_k)):
            pbias = pstile((128, 1), "pbias")
            nc.tensor.matmul(pbias[:], w_raw[0][:], tb_t[0][:],
                             start=True, stop=False)
            nc.tensor.matmul(pbias[:], w_raw[1][:], tb_t[1][:],
                             start=False, stop=True)
            nc.vector.tensor_add(out=out_b[:], in0=pbias[:], in1=host_b[:])
        for (cols, host_b, out_b) in ((slice(0, 64), bvA_sb, bias_vA),
                                      (slice(64, 128), bvB_sb, bias_vB)):
            pbias = pstile((64, 1), "pbiasv")
            nc.tensor.matmul(pbias[:], wv_sb[0][:, cols], tb_t[0][:],
                             start=True, stop=False)
            nc.tensor.matmul(pbias[:], wv_sb[1][:, cols], tb_t[1][:],
                             start=False, stop=True)
            nc.vector.tensor_add(out=out_b[:], in0=pbias[:], in1=host_b[:])

        # ---- q/k projection (k zero-padded per head to K=128) ----
        q_sb = stile((128, L), BF16, "q_sb")
        kp = {h: stile((128, L), BF16, f"kp{h}") for h in (0, 1)}
        zsrc = zeros_d[:]
        for h in (0, 1):
            zpad = bass.AP(tensor=zsrc.tensor, offset=zsrc.offset,
                           ap=[[0, 64]] + list(zsrc.ap)[1:])
            dst = kp[h][64:128, :] if h == 0 else kp[h][0:64, :]
            nc.sync.dma_start(out=dst, in_=zpad)
        for n in range(L // 512):
            nsl = slice(512 * n, 512 * (n + 1))
            for (wlist, kind) in ((wqs, "q"), (wks, "k")):
                pqkv = pstile((128, 512), f"pqkv{kind}")
                nc.tensor.matmul(pqkv[:], wlist[0][:], x_sb[0][:, nsl],
                                 start=True, stop=False)
                nc.tensor.matmul(pqkv[:], wlist[1][:], x_sb[1][:, nsl],
                                 start=False, stop=True)
                if kind == "q":
                    nc.vector.tensor_scalar_add(
                        out=q_sb[:, nsl], in0=pqkv[:], scalar1=bias_q[:])
                else:
                    nc.vector.tensor_scalar_add(
                        out=kp[0][0:64, nsl], in0=pqkv[0:64, :],
                        scalar1=bias_k[0:64, :])
                    nc.vector.tensor_scalar_add(
                        out=kp[1][64:128, nsl], in0=pqkv[64:128, :],
                        scalar1=bias_k[64:128, :])

        # ---- vT computed directly: lhsT = x tile, rhs = wv ----
        # vT[h]: (128=l, NJ, 65); col 64 of each j-tile = 1 (denominator)
        vT = {h: stile((128, NJ, 65), BF16, f"vT{h}") for h in (0, 1)}
        for h in (0, 1):
            nc.sync.dma_start(
                out=vT[h][:, :, 64:65],
                in_=ones_d[:, 0:NJ].rearrange("p (j o) -> p j o", o=1))
        for j in range(NJ):
            pvt = pstile((128, 128), "pvt")
            nc.tensor.matmul(pvt[:], x_sb[0][:, 128 * j:128 * (j + 1)],
                             wvs[0][:], start=True, stop=False)
            nc.tensor.matmul(pvt[:], x_sb[1][:, 128 * j:128 * (j + 1)],
                             wvs[1][:], start=False, stop=True)
            for h in (0, 1):
                nc.vector.tensor_copy(out=vT[h][:, j, 0:64],
                                      in_=pvt[:, 64 * h:64 * h + 64])

        # ---- attention ----
        oT = {h: stile((64, L), BF16, f"oT{h}") for h in (0, 1)}
        out_sb = [stile((128, L), F32, f"out_sb{m}") for m in range(2)]

        for i in range(NI):
            isl = slice(IBLK * i, IBLK * (i + 1))
            po = {h: ps.tile([65, IBLK], F32, name=f"oacc{h}", tag=f"oacc{h}",
                             bufs=1) for h in (0, 1)}
            for j in range(NJ):
                jsl = slice(128 * j, 128 * (j + 1))
                psc = {h: ps.tile([128, IBLK], F32, name=f"sc{h}",
                                  tag=("scA", "scB")[h], bufs=1)
                       for h in (0, 1)}
                for h in (0, 1):
                    for u in range(IBLK // 512):
                        usl = slice(IBLK * i + 512 * u,
                                    IBLK * i + 512 * (u + 1))
                        nc.tensor.matmul(
                            psc[h][:, 512 * u:512 * (u + 1)],
                            kp[h][:, jsl], q_sb[:, usl],
                            start=True, stop=True)
                    nc.scalar.activation(
                        out=(e := work.tile([128, IBLK], BF16, name=f"e{h}",
                                            tag=f"e{h}", bufs=2))[:],
                        in_=psc[h][:], func=AF.Exp, scale=1.0)
                    for u in range(IBLK // 512):
                        nc.tensor.matmul(
                            po[h][:, 512 * u:512 * (u + 1)],
                            vT[h][:, j, :], e[:, 512 * u:512 * (u + 1)],
                            start=(j == 0), stop=(j == NJ - 1))
            # epilogue: copy accumulator out of PSUM fast, then normalize
            for h, bias_v in ((0, bias_vA), (1, bias_vB)):
                po_sb = work.tile([65, IBLK], F32, name="po_sb", tag="po_sb",
                                  bufs=2)
                nc.vector.tensor_copy(out=po_sb[:], in_=po[h][:])
                recip = work.tile([65, IBLK], BF16, name="recip", tag="recip",
                                  bufs=2)
                with nc.allow_low_precision(reason="bf16 recip"):
                    nc.vector.reciprocal(out=recip[64:65, :],
                                         in_=po_sb[64:65, :])
                prep = ps.tile([64, IBLK], F32, name="prep",
                               tag=("oacc0", "oacc1")[h], bufs=1)
                for u in range(IBLK // 512):
                    ul = slice(512 * u, 512 * (u + 1))
                    nc.tensor.matmul(prep[:, ul], ones[64:65, 0:64],
                                     recip[64:65, ul], start=True, stop=True)
                rep_sb = work.tile([64, IBLK], F32, name="rep", tag="rep",
                                   bufs=2)
                nc.vector.tensor_copy(out=rep_sb[:], in_=prep[:])
                with nc.allow_low_precision(reason="bf16 oT"):
                    nc.vector.tensor_mul(out=oT[h][:, isl],
                                         in0=po_sb[0:64, :], in1=rep_sb[:])
                    nc.vector.tensor_scalar_add(
                        out=oT[h][:, isl], in0=oT[h][:, isl],
                        scalar1=bias_v[:])
            # proj for this i-block
            for m in range(2):
                msl = slice(128 * m, 128 * (m + 1))
                pp = pstile((128, IBLK), "pp")
                for u in range(IBLK // 512):
                    ul = slice(512 * u, 512 * (u + 1))
                    uabs = slice(IBLK * i + 512 * u, IBLK * i + 512 * (u + 1))
                    nc.tensor.matmul(pp[:, ul], wpA[:, msl], oT[0][:, uabs],
                                     start=True, stop=False)
                    nc.tensor.matmul(pp[:, ul], wpB[:, msl], oT[1][:, uabs],
                                     start=False, stop=True)
                nc.vector.tensor_copy(out=out_sb[m][:, isl], in_=pp[:])
                nc.sync.dma_start(out=part_d[msl, isl], in_=out_sb[m][:, isl])

    _split_waits(nc)
    return nc


def _host_prep(inputs, L):
    x = np.asarray(inputs["x"], dtype=np.float32)
    gn_w = np.asarray(inputs["gn_w"], dtype=np.float32)
    gn_b = np.asarray(inputs["gn_b"], dtype=np.float32)
    qkv_w = np.asarray(inputs["qkv_w"], dtype=np.float32)
    qkv_b = np.asarray(inputs["qkv_b"], dtype=np.float32)
    proj_w = np.asarray(inputs["proj_w"], dtype=np.float32)

    gsel = np.zeros((128, 16), np.float32)
    for cl in range(128):
        gsel[cl, cl // 8] = 1.0
    gbc = np.ascontiguousarray(gsel.T)
    ones = np.ones((128, 64), NPBF16)
    zeros = np.zeros((1, L), NPBF16)

    def bf(a):
        return np.ascontiguousarray(np.asarray(a, np.float32).astype(NPBF16))

    in_maps = []
    for c in range(NCORES):
        b, hp = c // 2, c % 2
        xb = np.ascontiguousarray(x[b].reshape(C, L))
        slq = slice(128 * hp, 128 * (hp + 1))
        slk = slice(256 + 128 * hp, 256 + 128 * (hp + 1))
        slv = slice(512 + 128 * hp, 512 + 128 * (hp + 1))
        wqT = np.ascontiguousarray(qkv_w[slq].T) * 0.125
        wkT = np.ascontiguousarray(qkv_w[slk].T)
        wvT = np.ascontiguousarray(qkv_w[slv].T)
        wpT = np.ascontiguousarray(proj_w[:, 128 * hp:128 * (hp + 1)].T)
        bq = qkv_b[slq].reshape(128, 1) * 0.125
        bk = qkv_b[slk].reshape(128, 1)
        bv = qkv_b[slv].reshape(128, 1)
        m = {
            "x0": bf(xb[0:128]),
            "x1": bf(xb[128:256]),
            "xf0": np.ascontiguousarray(xb[0:128]),
            "xf1": np.ascontiguousarray(xb[128:256]),
            "wq0": bf(wqT[0:128]),
            "wq1": bf(wqT[128:256]),
            "wk0": bf(wkT[0:128]),
            "wk1": bf(wkT[128:256]),
            "wv0": bf(wvT[0:128]),
            "wv1": bf(wvT[128:256]),
            "wpA": bf(wpT[0:64]),
            "wpB": bf(wpT[64:128]),
            "bq": np.ascontiguousarray(bq),
            "bk": np.ascontiguousarray(bk),
            "bvA": np.ascontiguousarray(bv[0:64]),
            "bvB": np.ascontiguousarray(bv[64:128]),
            "gnw0": np.ascontiguousarray(gn_w[0:128].reshape(128, 1)),
            "gnw1": np.ascontiguousarray(gn_w[128:256].reshape(128, 1)),
            "gnb0": np.ascontiguousarray(gn_b[0:128].reshape(128, 1)),
            "gnb1": np.ascontiguousarray(gn_b[128:256].reshape(128, 1)),
            "gsel": gsel,
            "gbc": gbc,
            "ones_in": ones,
            "zeros_in": zeros,
        }
        in_maps.append(m)
    return in_maps


def _run(inputs, trace=False):
    L = _L
    key = (L, _IBLK)
    if key not in _cache:
        _cache[key] = _build(L, _IBLK)
    nc = _cache[key]
    in_maps = _host_prep(inputs, L)
    res = run_bass_kernel_spmd(nc, in_maps, core_ids=list(range(NCORES)),
                               trace=trace)
    x = np.asarray(inputs["x"], dtype=np.float32)
    proj_b = np.asarray(inputs["proj_b"], dtype=np.float32)
    out = np.empty((B, C, L), np.float32)
    for b in range(B):
        out[b] = (res.results[2 * b]["part"] + res.results[2 * b + 1]["part"]
                  + x[b].reshape(C, L) + proj_b[:, None])
    return out.reshape(B, C, x.shape[2], x.shape[3]).astype(np.float32), res


def kernel(**inputs):
    out, _ = _run(inputs, trace=False)
    return out
